# revision 47
# baseline (speedup 1.0000x reference)
"""Causal self-attention (B=2, T=2048, C=1024, NH=16) on 8 TRN2 NeuronCores.

Sharding: pure head-parallel — core j owns heads {2j, 2j+1} for BOTH batches.
Each core computes qkv (transposed layouts) for its heads over all 4096 rows,
runs causal attention for its 4 (batch, head) instances, then the cores
exchange attention outputs with two 8-way AllToAlls (one per head) so that
core j ends up with all 1024 channels for global rows [512j, 512j+512).  The
projection is row-parallel and the host concatenates the per-core slices.

Final design notes (each point empirically trace-driven):
- qT/kT stored f16; score matmuls on f16 operands (fp32r moving operands
  stream ~2x slower per column; f16 measured 27us faster, same rel err).
- V computed directly in natural [keys, hd] layout (x tile stationary,
  w_v moving) — no PE transposes, no ACT staging copies.
- Phase 1 and head-0 attention are wave-interleaved (chunk rc-1 is emitted
  after qkv chunk rc) so the exp stream on ScalarE overlaps the qkv GEMM
  and the PE never idles long enough for HAM to re-throttle the clock.
  Early head-1 chunks ride in the late waves.
- x tiles are prefetched 3 row-chunks deep (bufs=24) from a host-pre-tiled
  layout; with bufs=8 each rc stalled ~3-6us on DMA and the PE clock
  dropped to K=4/8 half-rate.
- Causal masking off the critical path: score matmuls always start=True,
  exp on the live [s0:512] range only, diagonal blocks zero-filled after
  the exp by affine_select on GpSimd.
- Softmax: ones-columns packed next to V replicate the denominator into
  PSUM rows 64..127; 64-lane fast reciprocal on DVE.
- Projection: w_proj is loaded twice with head-half row interleavings so
  both per-head passes contract K=128 (cc-pairs); the h=0 pass overlaps
  the second AllToAll's flight and each output tile's copy + store DMA
  cascades off its final accumulation.  The exchange-result pulls issue
  on the Scalar engine's hardware-DGE queue so their completion-sem waits
  cannot head-of-line block the staging DMAs on the sync queue.
- A tiny AllReduce barrier fires mid-waves (its ~15us flight hides under
  compute): it pre-syncs the cores and the collective stream, collapsing
  the first AllToAll's rendezvous delay from ~11.5us to ~1.1us and
  shortening its flight (ragged-start mesh exchange) — the whole endgame
  shifts ~13us earlier.
- Dead ends (measured): fp8 anywhere on the linear path (2-4% error vs
  2e-2 budget), K=64 proj splits (2x streaming), PE "warmer" matmuls
  (overshoot the idle window when flights run short), w_proj loads inside
  the waves (+10us, DMA contention), sub-8-core AllToAlls (mesh needs >4
  cores).  The second AllToAll's doorbell trails the first's completion
  by ~10-12us (NRT straight-line ordering + completion-sem propagation),
  so the endgame floor is first-flight-end + that gap + second flight +
  one proj pass.
"""

import sys

sys.path.insert(0, "/opt/trn_rl_repo")

import numpy as np

import concourse.bass as bass
import concourse.mybir as mybir
from concourse import bacc, tile
from concourse import bass_utils

B, T, C = 2, 2048, 1024
NH, HD = 16, 64
R = B * T                 # 4096 global rows
P = 128
NCORES = 8
SCALE = 0.125             # 1/sqrt(HD)
CC = C // P               # 8 contraction chunks
RC = 8                    # row chunks of 512
RCH = R // RC             # 512
KT = T // P               # 16 k-tiles of 128 per batch

f32 = mybir.dt.float32
f16 = mybir.dt.float16

_PROGRAM = None


def _build_program(repeat=1, collective="a2a", num_devices=NCORES):
    nc = bacc.Bacc("TRN2", target_bir_lowering=False, debug=False,
                   num_devices=num_devices)
    # xT host-pre-tiled to [cc, rc, ci, t] so every x DMA is one contiguous
    # 128KB block (1KB strided lines measured ~3x slower)
    xT_ap = nc.dram_tensor("xT", [CC * RC * P, RCH], f16,
                           kind="ExternalInput").ap()
    # wqkv host-pre-swizzled to [ci, cc, 3*P] (contiguous 6KB lines)
    wqkv_ap = nc.dram_tensor("wqkv", [P, CC * 3 * P], f16,
                             kind="ExternalInput").ap()
    wproj_ap = nc.dram_tensor("wproj", [C, C], f16, kind="ExternalInput").ap()
    outT_ap = nc.dram_tensor("outT", [C, RCH], f16, kind="ExternalOutput").ap()

    with tile.TileContext(nc) as tc:
        for _rep in range(repeat):
            _emit_body(tc, nc, xT_ap, wqkv_ap, wproj_ap, outT_ap, collective)

    nc.compile()
    return nc


def _emit_body(tc, nc, xT_ap, wqkv_ap, wproj_ap, outT_ap, collective="a2a"):
    Exp = mybir.ActivationFunctionType.Exp
    with tc.tile_pool(name="const", bufs=1) as const, \
         tc.tile_pool(name="wp", bufs=1) as wpp, \
         tc.tile_pool(name="qkv", bufs=1) as qkvp, \
         tc.tile_pool(name="vo", bufs=1) as vop, \
         tc.tile_pool(name="yt", bufs=1) as ytp, \
         tc.tile_pool(name="ytm", bufs=8) as ytmp, \
         tc.tile_pool(name="dram", bufs=1, space="DRAM") as dram:

        # ---- constants -------------------------------------------------
        ones = const.tile([P, 1], f32)
        nc.gpsimd.memset(ones[:], 1.0)
        z64 = const.tile([64, P], f16)
        nc.gpsimd.memset(z64[:], 0.0)

        # w_proj loaded twice, rows re-interleaved per head half so the
        # proj passes contract cc-PAIRS at K=128:
        #   wp_h[h][m] partitions = [cc=2m head-h chans | cc=2m+1 head-h]
        wp_h = [[wpp.tile([P, C], f16, name=f"wp{h}_{m}") for m in range(4)]
                for h in range(2)]
        wproj_r = wproj_ap.rearrange("(cc hh r) n -> hh cc r n", cc=CC,
                                     hh=2, r=64)
        qT = qkvp.tile([P, R], f16, name="qT")
        kT = qkvp.tile([P, R], f16, name="kT")
        # [V_h0 | 1*64 | V_h1 | 1*64] per k-tile: the 64 ones-columns make the
        # PV matmul replicate the softmax denominator into PSUM rows 64..127,
        # so no partition-broadcast is needed for the normalization.
        vo = vop.tile([P, 2 * KT, 256], f16)
        yT = ytp.tile([P, R], f16)

        a2a_halves = [
            (dram.tile([NCORES * 64, RCH], f16, name=f"a2a_in{i}"),
             dram.tile([NCORES * 64, RCH], f16, name=f"a2a_out{i}"))
            for i in range(2)
        ]
        # tiny mid-wave barrier payload (values unused)
        bar_in = dram.tile([P, 4], f32, name="bar_in")
        bar_out = dram.tile([P, 4], f32, name="bar_out")
        # ytm2[h][m]: [cc=2m head-h chans | cc=2m+1 head-h chans] x my rows
        ytm2 = [[ytmp.tile([P, RCH], f16, tag="ytm", name=f"ytm{h}_{m}")
                 for m in range(4)] for h in range(2)]

        with tc.tile_pool(name="blk", bufs=3, space="PSUM") as ps_blk, \
             tc.tile_pool(name="misc", bufs=2, space="PSUM") as ps_misc, \
             tc.tile_pool(name="wq", bufs=1) as wqp, \
             tc.tile_pool(name="xt", bufs=24) as xtp, \
             tc.tile_pool(name="expp", bufs=16) as expp, \
             tc.tile_pool(name="small", bufs=6) as smallp:

            wq = wqp.tile([P, CC, 3 * P], f16)
            nc.sync.dma_start(wq[:], wqkv_ap.rearrange("ci (co n) -> ci co n",
                                                       co=CC))
            # ones columns of vo
            nc.vector.tensor_copy(vo[:, :, 64:128],
                                  ones[:, None, :].to_broadcast((P, 2 * KT, 64)))
            nc.vector.tensor_copy(vo[:, :, 192:256],
                                  ones[:, None, :].to_broadcast((P, 2 * KT, 64)))

            def phase1_rc(rc):
                xts = []
                for cc in range(CC):
                    xt = xtp.tile([P, RCH], f16, tag="xt", name="xt")
                    blk = (cc * RC + rc) * P
                    nc.sync.dma_start(xt[:], xT_ap[blk:blk + P, :])
                    xts.append(xt)
                pairA = ps_blk.tile([P, 2 * RCH], f32, tag="blk", name="pairA")
                for ct in range(2):
                    ps = pairA[:, RCH * ct:RCH * (ct + 1)]
                    for cc in range(CC):
                        nc.tensor.matmul(ps, wq[:, cc, P * ct:P * (ct + 1)],
                                         xts[cc][:], start=(cc == 0),
                                         stop=(cc == CC - 1))
                    dst = qT if ct == 0 else kT
                    nc.vector.tensor_copy(dst[:, RCH * rc:RCH * (rc + 1)], ps)
                # V directly in natural [keys, hd] layout: stationary is the
                # x tile itself (contraction over channels), moving is w_v.
                vps = ps_misc.tile([P, RCH], f32, tag="misc", name="vps")
                for s in range(RCH // P):
                    for cc in range(CC):
                        nc.tensor.matmul(vps[:, P * s:P * (s + 1)],
                                         xts[cc][:, P * s:P * (s + 1)],
                                         wq[:, cc, 2 * P:3 * P],
                                         start=(cc == 0), stop=(cc == CC - 1))
                for s in range(RCH // P):
                    kt32 = 4 * rc + s  # global k-tile index (0..31)
                    nc.vector.tensor_copy(vo[:, kt32, 0:64],
                                          vps[:, P * s:P * s + 64])
                    nc.vector.tensor_copy(vo[:, kt32, 128:192],
                                          vps[:, P * s + 64:P * (s + 1)])

            def attn_chunk(h, g, qc, dn_on_act=False):
                pr = 64 * h
                qoff = T * g + RCH * qc
                nkt = 4 * qc + 4
                exps = []
                for kt0 in range(0, nkt, 2):   # paired k-tiles
                    pair = ps_blk.tile([P, 2 * RCH], f32, tag="blk",
                                       name="pair")
                    es = expp.tile([P, 2 * RCH], f16, tag="ep", name="ep")
                    dpair = []
                    for j, kt in enumerate((kt0, kt0 + 1)):
                        koff = T * g + P * kt
                        d = kt - 4 * qc  # diagonal offset (>=0: diagonal)
                        s0 = P * d if d > 0 else 0
                        dpair.append((d, s0))
                        nc.tensor.matmul(
                            pair[:, RCH * j + s0:RCH * (j + 1)],
                            kT[pr:pr + 64, koff:koff + P],
                            qT[pr:pr + 64, qoff + s0:qoff + RCH],
                            start=True, stop=True, skip_group_check=True)
                    if dpair[0][0] < 0 and dpair[1][0] < 0:
                        # both off-diagonal: one 1024-wide exp
                        nc.scalar.activation(es[:, :], pair[:, :],
                                             Exp, scale=SCALE)
                    else:
                        for j in range(2):
                            d, s0 = dpair[j]
                            nc.scalar.activation(
                                es[:, RCH * j + s0:RCH * (j + 1)],
                                pair[:, RCH * j + s0:RCH * (j + 1)],
                                Exp, scale=SCALE)
                            # zero the causal upper triangle (and the
                            # never-computed cols < s0) after the exp
                            nc.gpsimd.affine_select(
                                out=es[:, RCH * j:RCH * (j + 1)],
                                in_=es[:, RCH * j:RCH * (j + 1)],
                                compare_op=mybir.AluOpType.is_ge,
                                fill=0.0, base=-P * d, pattern=[[1, RCH]],
                                channel_multiplier=-1)
                    exps.append((es, dpair))
                psy = ps_misc.tile([P, RCH], f32, tag="misc", name="psy")
                for kt in range(nkt):
                    es, dpair = exps[kt // 2]
                    j = kt % 2
                    d, s0 = dpair[j]
                    nc.tensor.matmul(
                        psy[:, s0:RCH],
                        vo[:, KT * g + kt, 128 * h:128 * h + 128],
                        es[:, RCH * j + s0:RCH * (j + 1)], start=(kt == 0),
                        stop=(kt == nkt - 1), skip_group_check=True)
                # psy rows 64..127 hold the softmax denominator (vo ones
                # columns).  Stage through SBUF: the custom-DVE approx
                # reciprocal cannot read partition-shifted PSUM.
                dn = smallp.tile([64, RCH], f32, tag="dn", name="dn")
                if dn_on_act:
                    # tail chunks: ScalarE is idle once the exps drain, and
                    # the serialized dn->recip->mul chains of the last
                    # chunks on DVE measured ~7us past the last PE matmul,
                    # directly delaying the second exchange's doorbell
                    nc.scalar.copy(dn[:], psy[64:128, :])
                else:
                    nc.vector.tensor_copy(dn[:], psy[64:128, :])
                rcb = smallp.tile([64, RCH], f32, tag="recip", name="rcb")
                nc.vector.reciprocal_approx_fast(rcb[:], dn[:])
                nc.vector.tensor_mul(yT[pr:pr + 64, qoff:qoff + RCH],
                                     psy[0:64, :], rcb[:])
                if collective == "a2a":
                    # stage this finished slab into the exchange buffer
                    # while later chunks compute
                    s = 4 * g + qc
                    nc.sync.dma_start(
                        a2a_halves[h][0][64 * s:64 * (s + 1), :],
                        yT[pr:pr + 64, RCH * s:RCH * (s + 1)])

            def a2a(h):
                # exchange this head-half while other work computes.
                # unique_tensors: truthful (all collective buffers are
                # distinct dram tiles) — lets the runtime skip conservative
                # aliasing checks between the collectives.
                nc.gpsimd.collective_compute(
                    "AllToAll", mybir.AluOpType.bypass,
                    replica_groups=[list(range(NCORES))],
                    ins=[a2a_halves[h][0].opt()], outs=[a2a_halves[h][1].opt()],
                    unique_tensors="Yes")

            def a2a_pulls(h):
                # pull the arrived channel blocks into the proj staging
                # tiles (cc-pair packed).  Issued on the SCALAR engine's
                # hardware-DGE queue: these wait on the collective's
                # completion sem, and on the (FIFO) sync queue they
                # head-of-line blocked the h=1 staging DMAs — the second
                # trigger fired ~20us late (measured both with pulls
                # emitted before AND after the staging; the tile
                # scheduler reordered them ahead either way).
                # blocks 2m and 2m+1 are contiguous in the output, so one
                # 128KB DMA fills each cc-pair staging tile.  For the h=1
                # pulls the sync queue has drained, so split across both
                # hwdge queues; h=0 pulls stay off the sync queue (its
                # staging backlog is still draining then).
                for m in range(4):
                    eng = nc.sync if (h == 1 and m % 2 == 1) else nc.scalar
                    eng.dma_start(
                        ytm2[h][m][:],
                        a2a_halves[h][1][P * m:P * (m + 1), :])

            # ---- waves: phase-1 rc interleaved with h=0 attention chunks
            # (chunks run one wave behind so score matmuls never wait on
            # the same wave's qkv PSUM->SBUF copies)
            for rc in range(RC):
                phase1_rc(rc)
                if rc == 4 and collective == "a2a":
                    # re-sync the cores mid-waves (flight hides under
                    # compute): the first AllToAll's flight measured
                    # 25-31us vs 13-20us for the second on identical
                    # payloads — the delta is trigger-time core skew
                    # making the mesh exchange ragged
                    nc.gpsimd.collective_compute(
                        "AllReduce", mybir.AluOpType.add,
                        replica_groups=[list(range(NCORES))],
                        ins=[bar_in.opt()], outs=[bar_out.opt()])
                if rc > 0:
                    attn_chunk(0, (rc - 1) // 4, (rc - 1) % 4)
                if rc >= 4:
                    # all four g0 head-1 chunks ride along in the late
                    # waves, filling ScalarE slack and shortening the
                    # serial head-1 tail (so the second AllToAll fires
                    # earlier)
                    attn_chunk(1, 0, rc - 4)
            attn_chunk(0, 1, 3, dn_on_act=True)
            if collective == "p1":
                for ct in range(CC):
                    ot = const.tile([P, RCH], f16, tag="ot", name="ot")
                    nc.vector.tensor_copy(ot[:], qT[:, RCH * ct:RCH * (ct + 1)])
                    nc.sync.dma_start(outT_ap[P * ct:P * (ct + 1), :], ot[:])
                return
            if collective == "a2a":
                a2a(0)
            # proj weight loads: off the startup critical path, behind the
            # first exchange's staging in the DMA queue
            for h in range(2):
                for m in range(4):
                    for p2 in range(2):
                        nc.sync.dma_start(
                            wp_h[h][m][64 * p2:64 * (p2 + 1), :],
                            wproj_r[h, 2 * m + p2, :, :])

            # ---- head-1 attention (all g0 chunks already ran in the waves)
            for qc in range(4):
                attn_chunk(1, 1, qc, dn_on_act=(qc >= 1))
            if collective == "p2":
                for ct in range(CC):
                    ot = const.tile([P, RCH], f16, tag="ot", name="ot")
                    nc.vector.tensor_copy(ot[:], yT[:, RCH * ct:RCH * (ct + 1)])
                    nc.sync.dma_start(outT_ap[P * ct:P * (ct + 1), :], ot[:])
                return
            if collective == "a2a":
                a2a(1)
                a2a_pulls(0)
                a2a_pulls(1)
            else:
                # debug path: local copy emulating the exchange
                a2a_in = dram.tile([C, RCH], f16, name="a2a_in_dbg")
                a2a_out = dram.tile([C, RCH], f16, name="a2a_out_dbg")
                for i in range(NCORES):
                    nc.sync.dma_start(a2a_in[P * i:P * (i + 1), :],
                                      yT[:, RCH * i:RCH * (i + 1)])
                nc.sync.dma_start(a2a_out[:], a2a_in[:])
                for cc in range(CC):
                    nc.sync.dma_start(
                        ytm2[cc % 2][cc // 2][64 * (cc % 2):64 * (cc % 2) + 64, :],
                        a2a_out[P * cc:P * (cc + 1), :])

        # ---- phase 3: projection (two K=128 cc-pair passes; pass h=0
        # overlaps the second AllToAll's flight) ------------------------
        with tc.tile_pool(name="pj", bufs=4, space="PSUM") as ps_pj, \
             tc.tile_pool(name="outsb", bufs=4) as outsbp:
            pps = [ps_pj.tile([P, 2 * RCH], f32, tag="pj", name="pp")
                   for _ in range(4)]
            for h in range(2):
                for ct in range(CC):
                    pp = pps[ct // 2]
                    jj = ct % 2
                    for m in range(4):
                        nc.tensor.matmul(
                            pp[:, RCH * jj:RCH * (jj + 1)],
                            wp_h[h][m][:, P * ct:P * (ct + 1)],
                            ytm2[h][m][:],
                            start=(h == 0 and m == 0),
                            stop=(h == 1 and m == 3),
                            skip_group_check=True)
                    if h == 1:
                        # cascade the output per ct so copies + out-DMA
                        # overlap the remaining proj matmuls
                        ot = outsbp.tile([P, RCH], f16, name="oto")
                        if ct % 2 == 0:
                            nc.vector.tensor_copy(
                                ot[:], pp[:, RCH * jj:RCH * (jj + 1)])
                        else:
                            nc.scalar.copy(ot[:], pp[:, RCH * jj:RCH * (jj + 1)])
                        nc.sync.dma_start(outT_ap[P * ct:P * (ct + 1), :], ot[:])


def _get_program():
    global _PROGRAM
    if _PROGRAM is None:
        _PROGRAM = _build_program()
    return _PROGRAM


def make_in_maps(x, w_qkv, w_proj):
    """Host-side sharding: build the 8 per-core input maps (f16 payloads)."""
    x = np.asarray(x, dtype=np.float32)
    w_qkv = np.asarray(w_qkv, dtype=np.float32)
    w_proj = np.asarray(w_proj, dtype=np.float32)
    xT = x.reshape(R, C).T.astype(np.float16)
    # pre-tile to [cc, rc, ci, t] so each device-side x DMA is one
    # contiguous 128KB block
    xTt = np.ascontiguousarray(
        xT.reshape(CC, P, RC, RCH).transpose(0, 2, 1, 3)).reshape(
            CC * RC * P, RCH)
    w_proj16 = np.ascontiguousarray(w_proj).astype(np.float16)
    in_maps = []
    for j in range(NCORES):
        h0 = 2 * j * HD                                     # first head col
        wq = w_qkv[:, h0:h0 + 2 * HD]
        wk = w_qkv[:, C + h0:C + h0 + 2 * HD]
        wv = w_qkv[:, 2 * C + h0:2 * C + h0 + 2 * HD]
        wshard = np.concatenate([wq, wk, wv], axis=1).astype(np.float16)
        # pre-swizzle to [ci, co, n] (contiguous 6KB lines per partition)
        wshard = np.ascontiguousarray(
            wshard.reshape(CC, P, 3 * P).transpose(1, 0, 2)).reshape(
                P, CC * 3 * P)
        in_maps.append({"xT": xTt, "wqkv": wshard, "wproj": w_proj16})
    return in_maps


def assemble(results):
    """Host-side unshard: concatenate per-core transposed row slices."""
    y = np.empty((R, C), dtype=np.float32)
    for j in range(NCORES):
        y[RCH * j:RCH * (j + 1), :] = results[j]["outT"].T
    return y.reshape(B, T, C)


def kernel(x, w_qkv, w_proj):
    nc = _get_program()
    in_maps = make_in_maps(x, w_qkv, w_proj)
    res = bass_utils.run_bass_kernel_spmd(nc, in_maps,
                                          core_ids=list(range(NCORES)))
    return assemble(res.results)
